# revision 15
# baseline (speedup 1.0000x reference)
"""DIN attention-pooling kernel for Trainium2 (8 NeuronCores, data-parallel over batch).

Math (per batch row b):
  din = [q, f, q-f, q*f] @ W1 + b1  ->  sigmoid -> @ W2 + b2 -> masked softmax over T
  out = softmax(scores) @ facts

Reformulation used on-chip:
  z[b,t,:] = f[b,t,:] @ Wb_b + qc[b,:]      with Wb_b = (W1b - W1c) + diag(q_b) @ W1d
  qc[b,:]  = q_b @ (W1a + W1c) + b1         (host-precomputed, folded into sigmoid bias)
  scores   = sigmoid(z) @ W2 + b2
  attn     = exp(scores) * mask / sum       (no max-subtraction needed: |scores| <= sum|W2|)
  out      = attn @ facts

Per-core layout (B_local=512, groups of G=32 rows, 4 packs of 8 rows each):
  slot mapping: b_local = 32*grp + 8*i + 4*c + g   (i: pack 0..3, c: col-half 0..1, g: 0..3)
  facts loaded once (fp32 HBM -> bf16 SBUF cast-DMA) as t-strips S0 [128t, 32*128],
  S1 [128t(72 valid), 32*128]; fT per row via DMA xbar transpose; z via per-row-weight
  matmuls (M=32 zero-padded) into a packed PSUM [128, 400]; sigmoid/exp on ACT; weighted
  sum via attn-column-stationary matmuls contracting t on the natural-layout strips.
"""

import sys

sys.path.insert(0, "/opt/trn_rl_repo")

from contextlib import ExitStack

import ml_dtypes
import numpy as np

import concourse.bass as bass
import concourse.tile as tile
from concourse import bacc, mybir
from concourse.bass_utils import run_bass_kernel_spmd


def _install_ntff_hook_shim():
    """Provide antenv.axon_hooks (missing in this image) so trace=True works."""
    import contextlib
    import ctypes
    import types

    if "antenv.axon_hooks" in sys.modules:
        return
    so_path = "/opt/axon/libaxon_pjrt.so"
    try:
        lib = ctypes.CDLL(so_path)
        if not hasattr(lib, "axon_start_nrt_profile"):
            return
    except OSError:
        return
    lib.axon_start_nrt_profile.argtypes = [ctypes.POINTER(ctypes.c_int64), ctypes.c_size_t]
    lib.axon_start_nrt_profile.restype = ctypes.c_int64
    lib.axon_stop_nrt_profile.argtypes = [ctypes.c_char_p]
    lib.axon_stop_nrt_profile.restype = ctypes.c_int64

    @contextlib.contextmanager
    def _hook(output_dir, device_ids):
        import jax

        jax.devices()
        if device_ids:
            ids = (ctypes.c_int64 * len(device_ids))(*device_ids)
            rc = lib.axon_start_nrt_profile(ids, len(device_ids))
        else:
            rc = lib.axon_start_nrt_profile(None, 0)
        if rc != 0:
            raise RuntimeError(f"axon_start_nrt_profile rc={rc}")
        try:
            yield
        finally:
            n = lib.axon_stop_nrt_profile(str(output_dir).encode())
            print(f"ntff profile: {n} file(s) -> {output_dir}", file=sys.stderr)

    mod = types.ModuleType("antenv.axon_hooks")
    mod.get_axon_ntff_profile_hook = lambda: _hook
    mod.set_axon_ntff_profile_hook = lambda h: None
    sys.modules["antenv.axon_hooks"] = mod


_install_ntff_hook_shim()

BF16 = ml_dtypes.bfloat16
F32 = np.float32

B, T, D, H = 4096, 200, 128, 16
NCORES = 8
BL = B // NCORES  # 512 rows per core
G = 32  # rows per group
NG = BL // G  # 16 groups
NPACK_G = 4  # packs per group (8 rows each)

AF = mybir.ActivationFunctionType
DT = mybir.dt

_PROG_CACHE = {}


def _build_program(b2_val: float):
    """Build the single-core Bass program (identical across the 8 cores)."""
    nc = bacc.Bacc("TRN2", target_bir_lowering=False, debug=False, num_devices=1)

    facts_d = nc.dram_tensor("facts", [BL, T, D], DT.float32, kind="ExternalInput").ap()
    qT_d = nc.dram_tensor("qT", [D, BL], DT.bfloat16, kind="ExternalInput").ap()
    bias_d = nc.dram_tensor("bias_all", [128, 128], DT.bfloat16, kind="ExternalInput").ap()
    maskb_d = nc.dram_tensor("maskbias", [16, NG * 400], DT.bfloat16, kind="ExternalInput").ap()
    wf_d = nc.dram_tensor("Wf", [D, H], DT.bfloat16, kind="ExternalInput").ap()
    wp_d = nc.dram_tensor("Wp", [D, H], DT.bfloat16, kind="ExternalInput").ap()
    w2bd4_d = nc.dram_tensor("W2bd4", [4, D, 16], DT.bfloat16, kind="ExternalInput").ap()
    id16_d = nc.dram_tensor("id16", [16, 16], DT.bfloat16, kind="ExternalInput").ap()
    out_d = nc.dram_tensor("out", [BL, D], DT.float32, kind="ExternalOutput").ap()

    # out rows b = 32*grp + 8*i + 4*c + g  ->  [grp, c, g, i, d]
    out_r = out_d.rearrange("(q i c g) d -> q c g i d", i=4, c=2, g=4)

    with tile.TileContext(nc) as tc, ExitStack() as ctx:
        consts = ctx.enter_context(tc.tile_pool(name="consts", bufs=1))
        pers = ctx.enter_context(tc.tile_pool(name="pers", bufs=1))
        s0_pool = ctx.enter_context(tc.tile_pool(name="s0", bufs=3))
        ft_pool = ctx.enter_context(tc.tile_pool(name="ft", bufs=16))
        h_pool = ctx.enter_context(tc.tile_pool(name="h", bufs=3))
        e_pool = ctx.enter_context(tc.tile_pool(name="e", bufs=2))
        at_pool = ctx.enter_context(tc.tile_pool(name="at", bufs=8))
        o_pool = ctx.enter_context(tc.tile_pool(name="o", bufs=4))
        small = ctx.enter_context(tc.tile_pool(name="small", bufs=4))
        zps_pool = ctx.enter_context(tc.tile_pool(name="zps", bufs=2, space="PSUM"))
        sps_pool = ctx.enter_context(tc.tile_pool(name="sps", bufs=2, space="PSUM"))
        wps_pool = ctx.enter_context(tc.tile_pool(name="wps", bufs=2, space="PSUM"))

        # ---- one-time loads ----
        qT_s = consts.tile([D, BL], DT.bfloat16)
        nc.sync.dma_start(qT_s[:], qT_d)
        bias_s = consts.tile([128, 128], DT.bfloat16)
        nc.sync.dma_start(bias_s[:], bias_d)
        maskb_s = consts.tile([16, NG * 400], DT.bfloat16)
        nc.sync.dma_start(maskb_s[:], maskb_d)
        wf_s = consts.tile([D, H], DT.bfloat16)
        nc.sync.dma_start(wf_s[:], wf_d)
        wp_s = consts.tile([D, H], DT.bfloat16)
        nc.sync.dma_start(wp_s[:], wp_d)
        w2bd4_s = consts.tile([D, 64], DT.bfloat16)
        nc.sync.dma_start(
            w2bd4_s.rearrange("p (q h) -> p q h", q=4), w2bd4_d.rearrange("q p h -> p q h")
        )
        id16_s = consts.tile([16, 16], DT.bfloat16)
        nc.sync.dma_start(id16_s[:], id16_d)

        # ---- persistent double-buffered tiles ----
        s1_p = [pers.tile([128, G * 128], DT.bfloat16, tag=f"s1_{k}", name=f"s1_{k}") for k in range(2)]
        wb_p = [pers.tile([128, G * 32], DT.bfloat16, tag=f"wb_{k}", name=f"wb_{k}") for k in range(2)]
        for k in range(2):
            nc.vector.memset(s1_p[k][:], 0.0)  # rows 0:72 overwritten by DMA each group
            nc.vector.memset(wb_p[k][:], 0.0)  # cols 16..31 of each 32-block stay 0

        for grp in range(NG):
            b0 = grp * G
            s1 = s1_p[grp % 2]
            wb = wb_p[grp % 2]

            # ---- facts load (fp32 -> bf16 cast during DMA) ----
            s0 = s0_pool.tile([128, G * 128], DT.bfloat16)
            nc.gpsimd.dma_start(
                s0.rearrange("p (b d) -> p b d", b=G),
                facts_d[b0 : b0 + G, 0:128, :].rearrange("b t d -> t b d"),
            )
            nc.gpsimd.dma_start(
                s1[0:72, :].rearrange("p (b d) -> p b d", b=G),
                facts_d[b0 : b0 + G, 128:200, :].rearrange("b t d -> t b d"),
            )

            # ---- per-row weights: Wb_j = Wp * q_j + Wf (j = row in group) ----
            wb_blocks = wb.rearrange("p (j w) -> p j w", w=32)[:, :, 0:H]
            q_bc = qT_s[:, b0 : b0 + G].unsqueeze(2).broadcast_to([D, G, H])
            wf_bc = wf_s.unsqueeze(1).broadcast_to([D, G, H])
            wp_bc = wp_s.unsqueeze(1).broadcast_to([D, G, H])
            nc.vector.tensor_mul(wb_blocks, wp_bc, q_bc)
            nc.vector.tensor_add(wb_blocks, wb_blocks, wf_bc)

            # ---- E tile for this group: rows 4*i+g, cols c*256 + t ----
            e_t = e_pool.tile([16, 512], DT.bfloat16)
            e_3d = e_t.rearrange("p (c t) -> p c t", c=2)
            nc.vector.memset(e_3d[:, :, 200:256], 0.0)  # pads must be 0 for attnT chunk2
            sp16 = sps_pool.tile([16, 400], DT.float32)

            for i in range(NPACK_G):
                # ---- transposed facts for the 8 rows of this pack ----
                fts = []
                for c in range(2):
                    for g in range(4):
                        j = 8 * i + 4 * c + g
                        ft = ft_pool.tile([128, 256], DT.bfloat16)
                        nc.sync.dma_start_transpose(
                            ft[:, 0:128], s0[:, 128 * j : 128 * (j + 1)]
                        )
                        nc.sync.dma_start_transpose(
                            ft[:, 128:256], s1[:, 128 * j : 128 * (j + 1)]
                        )
                        fts.append(ft)

                # ---- z matmuls: zT packed [128, 400], slot (g,c) at rows 32g+0..15 ----
                zps = zps_pool.tile([128, 400], DT.float32)
                for c in range(2):
                    for g in range(4):
                        j = 8 * i + 4 * c + g
                        nc.tensor.matmul(
                            zps[32 * g : 32 * g + 32, 200 * c : 200 * (c + 1)],
                            lhsT=wb[:, 32 * j : 32 * j + 32],
                            rhs=fts[4 * c + g][:, 0:200],
                            start=True,
                            stop=True,
                            tile_position=(0, 32 * g),
                        )

                # ---- sigmoid (+qc bias) -> h packed bf16 [128, 400] ----
                h_t = h_pool.tile([128, 400], DT.bfloat16)
                n_glob = NPACK_G * grp + i
                for c in range(2):
                    nc.scalar.activation(
                        h_t[:, 200 * c : 200 * (c + 1)],
                        zps[:, 200 * c : 200 * (c + 1)],
                        AF.Sigmoid,
                        bias=bias_s[:, 2 * n_glob + c : 2 * n_glob + c + 1],
                        scale=1.0,
                    )

                # ---- scores: rows 4i..4i+3 of sp16 via zero-padded stationary ----
                nc.tensor.matmul(
                    sp16[:, :], lhsT=w2bd4_s[:, 16 * i : 16 * (i + 1)],
                    rhs=h_t[:, 0:400], start=(i == 0), stop=False,
                )

            # ---- mask add (-30000 on masked slots) + exp + row sums + normalize ----
            nc.tensor.matmul(
                sp16[:, :], lhsT=id16_s[:], rhs=maskb_s[:, 400 * grp : 400 * (grp + 1)],
                start=False, stop=True,
            )
            sums = small.tile([16, 2], DT.float32, tag="sums")
            sp16_3d = sp16.rearrange("p (c t) -> p c t", c=2)
            for c in range(2):
                nc.scalar.activation(
                    e_3d[:, c, 0:200], sp16_3d[:, c, :], AF.Exp,
                    bias=float(b2_val), scale=1.0,
                    accum_out=sums[:, c : c + 1],
                )
            rsum = small.tile([16, 2], DT.float32, tag="rsum")
            nc.vector.reciprocal(rsum[:], sums[:])
            nc.vector.tensor_mul(e_3d, e_3d, rsum.unsqueeze(2).broadcast_to([16, 2, 256]))

            # ---- attn transpose: [16, 128] -> [128, 16] per (c, t-chunk) ----
            at_t = {}
            for c in range(2):
                for tc_i in range(2):
                    at = at_pool.tile([128, 16], DT.bfloat16)
                    nc.sync.dma_start_transpose(
                        at[:], e_t[0:16, 256 * c + 128 * tc_i : 256 * c + 128 * (tc_i + 1)]
                    )
                    at_t[(c, tc_i)] = at

            # ---- weighted sum: out[b,:] = sum_t attn * facts ----
            wps = {c: wps_pool.tile([128, 512], DT.float32, tag=f"wps{c}", name=f"wps{c}") for c in range(2)}
            for c in range(2):
                for i in range(NPACK_G):
                    for g in range(4):
                        j = 8 * i + 4 * c + g
                        r = 4 * i + g
                        nc.tensor.matmul(
                            wps[c][32 * g : 32 * g + 1, 128 * i : 128 * (i + 1)],
                            lhsT=at_t[(c, 0)][:, r : r + 1],
                            rhs=s0[:, 128 * j : 128 * (j + 1)],
                            start=True,
                            stop=False,
                            tile_position=(0, 32 * g),
                        )
                        nc.tensor.matmul(
                            wps[c][32 * g : 32 * g + 1, 128 * i : 128 * (i + 1)],
                            lhsT=at_t[(c, 1)][0:72, r : r + 1],
                            rhs=s1[0:72, 128 * j : 128 * (j + 1)],
                            start=False,
                            stop=True,
                            tile_position=(0, 32 * g),
                        )

            # ---- copy full psum (rows 32g valid) + strided-row store ----
            for c in range(2):
                o_t = o_pool.tile([128, 512], DT.float32)
                nc.scalar.copy(o_t[:], wps[c][:])
                o_v = o_t.rearrange("(g u) f -> g u f", u=32)[:, 0, :]
                nc.sync.dma_start(out_r[grp, c], o_v.rearrange("g (i d) -> g i d", i=4))

    nc.finalize()
    return nc


def _host_prep(query, facts, mask, W1, b1, W2, b2):
    """Build per-core input maps (weight reformulation + layout permutations)."""
    W1 = W1.astype(np.float64)
    Wq = (W1[0:D] + W1[2 * D : 3 * D]).astype(F32)
    Wf = (W1[D : 2 * D] - W1[2 * D : 3 * D]).astype(F32)
    Wp = W1[3 * D : 4 * D].astype(F32)
    qc_all = (query.astype(np.float64) @ Wq.astype(np.float64)).astype(F32) + b1[None, :]

    # W2bd4[i][32g+j, 4i+g] = W2[j]; id16 = identity
    w2bd4 = np.zeros((4, 128, 16), dtype=F32)
    for i in range(4):
        for g in range(4):
            w2bd4[i, 32 * g : 32 * g + H, 4 * i + g] = W2[:, 0]
    w2bd4 = w2bd4.astype(BF16)
    id16 = np.eye(16, dtype=BF16)

    in_maps = []
    for core in range(NCORES):
        sl = slice(core * BL, (core + 1) * BL)
        q_c = query[sl]
        qc_c = qc_all[sl]
        mask_c = mask[sl]

        # bias_all[32*g + u, 2*n + c] = qc[8n+4c+g, u]
        bias_all = np.zeros((128, 128), dtype=F32)
        qc_r = qc_c.reshape(64, 2, 4, H)  # [n, c, g, h]
        for g in range(4):
            bias_all[32 * g : 32 * g + H, :] = (
                qc_r[:, :, g, :].reshape(128, H).T
            )
        # maskbias[4i+g, 400*grp + 200c + t] = 0 valid / -30000 masked
        m_r = mask_c.reshape(NG, 4, 2, 4, T)  # [grp, i, c, g, t]
        m_igc = np.transpose(m_r, (0, 1, 3, 2, 4)).reshape(NG, 16, 2 * T)  # [grp, 4i+g, (c t)]
        maskbias = np.where(m_igc, 0.0, -30000.0).astype(F32)
        maskbias = np.transpose(maskbias, (1, 0, 2)).reshape(16, NG * 400)

        in_maps.append(
            {
                "facts": np.ascontiguousarray(facts[sl]),
                "qT": np.ascontiguousarray(q_c.T).astype(BF16),
                "bias_all": bias_all.astype(BF16),
                "maskbias": maskbias.astype(BF16),
                "Wf": Wf.astype(BF16),
                "Wp": Wp.astype(BF16),
                "W2bd4": w2bd4,
                "id16": id16,
            }
        )
    return in_maps


def run(inputs: dict, trace: bool = False):
    """Run the kernel on 8 cores; returns (output [B, D] f32, BassKernelResults)."""
    b2_val = float(np.asarray(inputs["b2"]).reshape(-1)[0])
    key = ("prog", round(b2_val, 9))
    if key not in _PROG_CACHE:
        _PROG_CACHE[key] = _build_program(b2_val)
    nc = _PROG_CACHE[key]

    in_maps = _host_prep(
        inputs["query"], inputs["facts"], inputs["mask"].astype(bool),
        inputs["W1"], inputs["b1"], inputs["W2"], inputs["b2"],
    )
    res = run_bass_kernel_spmd(nc, in_maps, core_ids=list(range(NCORES)), trace=trace)
    out = np.concatenate([res.results[c]["out"] for c in range(NCORES)], axis=0)
    return out.astype(F32), res


def kernel(query, facts, mask, W1, b1, W2, b2) -> np.ndarray:
    out, _ = run(
        {"query": query, "facts": facts, "mask": mask, "W1": W1, "b1": b1,
         "W2": W2, "b2": b2}
    )
    return out
